# revision 1
# baseline (speedup 1.0000x reference)
"""IsoMaxPlus first-part kernel for Trainium2 (8 NeuronCores, SPMD).

Math (per point n, prototype k):
    xn = x / ||x||;  pn = p / ||p||
    d2[n,k] = ||xn||^2 + ||pn||^2 - 2 xn.pn  ~= 2 - 2 (x.pn)/||x||
    out[n,k] = -|s| * sqrt(d2)

Device dataflow per core (2 of 16 batches, channels on partitions):
    for each macro-tile of NF=1024 points:
      DMA x1,x2 [128, NF]  (C=256 split in two chunks)
      squares q1 (ACT), q2 (DVE)
      PE: g[19,NF]  = W1.T@x1 + W2.T@x2      (W = -2 * pn, fp32r)
          ss[1,NF]  = ones.T@q1 + ones.T@q2  (sum of squares)
      ACT: r = sqrt(ss); DVE: ri = 1/r; DMA: broadcast ri to [19,NF]
      DVE: t = g * ri_rep  (= -2 g / r)
      ACT: u = sqrt(s^2 * t + 2 s^2) = |s| sqrt(d2)
      DVE: o = -u ; DMA out
"""

import numpy as np

B, C, H, W = 16, 256, 128, 256
K = 19
NCORES = 8
BPC = B // NCORES          # batches per core
HW = H * W                 # 32768 points per batch
NF = 1024                  # points per macro-tile
NSUB = NF // 512           # matmul subtiles (PSUM bank limit: N<=512 fp32)
EPS = 1e-12


def _split_excess_waits(nc):
    """Walrus limits the sync-wait slots per ISA instruction (TensorTensor
    takes only 1, DMAs 2, ...). Hoist excess waits onto same-engine NoOps
    inserted right before the instruction — engines execute in order, so
    all waits still complete before the instruction runs."""
    import bass_rust
    import concourse.mybir as mybir

    limits = {}
    default_limit = 1
    skip = {"InstEventSemaphore", "InstNoOp", "InstCall",
            "InstUnconditionalBranch", "InstISA", "InstRegisterMove"}
    nseq = 0
    for fn in nc.m.functions:
        for blk in fn.blocks:
            new = []
            for I in blk.instructions:
                tn = type(I).__name__
                si = I.sync_info
                waits = list(si.on_wait) if si else []
                lim = limits.get(tn, default_limit)
                if tn in skip or len(waits) <= lim:
                    new.append(I)
                    continue
                keep = waits[-lim:]
                excess = waits[:-lim]
                for w in excess:
                    nop = mybir.InstNoOp(name=f"{I.name}-w{nseq}", ins=[], outs=[])
                    nseq += 1
                    nop.engine = I.engine
                    nop.sync_info = bass_rust.SyncInfo(on_wait=[w], on_update=[])
                    new.append(nop)
                I.sync_info = bass_rust.SyncInfo(
                    on_wait=keep, on_update=list(si.on_update) if si else []
                )
                new.append(I)
            blk.instructions = new
    return nc


def build_program(bpc=BPC, hw=HW, nf=NF, split_waits=True):
    from contextlib import ExitStack

    import concourse.bass as bass
    import concourse.mybir as mybir
    import concourse.tile as tile

    f32 = mybir.dt.float32
    f32r = mybir.dt.float32r
    nsub = nf // 512
    nmacro = hw // nf

    nc = bass.Bass()
    feat = nc.declare_dram_parameter("features", [bpc, C, hw], f32, isOutput=False)
    wn = nc.declare_dram_parameter("wneg2", [128, 2, K], mybir.dt.bfloat16, isOutput=False)
    sv = nc.declare_dram_parameter("svec", [K, 1], f32, isOutput=False)
    bv = nc.declare_dram_parameter("bvec", [K, 1], f32, isOutput=False)
    nv = nc.declare_dram_parameter("negv", [K, 1], f32, isOutput=False)
    out = nc.declare_dram_parameter("out", [bpc, K, hw], f32, isOutput=True)

    with ExitStack() as ctx:
        tc = ctx.enter_context(tile.TileContext(nc))
        singles = ctx.enter_context(tc.tile_pool(name="singles", bufs=1))
        xpool = ctx.enter_context(tc.tile_pool(name="x", bufs=8))
        xbpool = ctx.enter_context(tc.tile_pool(name="xb", bufs=3))
        sqpool = ctx.enter_context(tc.tile_pool(name="sq", bufs=3))
        gpool = ctx.enter_context(tc.tile_pool(name="g", bufs=2, space="PSUM"))
        spool = ctx.enter_context(tc.tile_pool(name="ss", bufs=2, space="PSUM"))
        rpool = ctx.enter_context(tc.tile_pool(name="r", bufs=4))
        opool = ctx.enter_context(tc.tile_pool(name="o", bufs=3))

        bf16 = mybir.dt.bfloat16
        w_s = singles.tile([128, 2, K], bf16)
        nc.sync.dma_start(out=w_s, in_=wn[:, :, :])
        # ones replicated K wide: the ssq matmul then emits sum-of-squares
        # already broadcast across the K output partitions (no bcast needed)
        ones_s = singles.tile([128, K], bf16)
        nc.vector.memset(ones_s, 1.0)
        sv_s = singles.tile([K, 1], f32)
        nc.sync.dma_start(out=sv_s, in_=sv[:, :])
        bv_s = singles.tile([K, 1], f32)
        nc.sync.dma_start(out=bv_s, in_=bv[:, :])
        nv_s = singles.tile([K, 1], f32)
        nc.sync.dma_start(out=nv_s, in_=nv[:, :])

        for b in range(bpc):
            for m in range(nmacro):
                h0 = m * nf
                xt = xpool.tile([128, 2, nf], f32, tag="xt")
                nc.sync.dma_start(
                    out=xt,
                    in_=feat[b, :, h0 : h0 + nf].rearrange(
                        "(j c) n -> c j n", c=128
                    ),
                )

                # f32 -> bf16 casts on the (otherwise idle) gpsimd engine
                x1 = xbpool.tile([128, nf], bf16, tag="x1")
                nc.gpsimd.tensor_copy(out=x1, in_=xt[:, 0, :])
                x2 = xbpool.tile([128, nf], bf16, tag="x2")
                nc.gpsimd.tensor_copy(out=x2, in_=xt[:, 1, :])

                q1 = sqpool.tile([128, nf], bf16, tag="q1")
                nc.vector.tensor_mul(out=q1, in0=x1, in1=x1)
                q2 = sqpool.tile([128, nf], bf16, tag="q2")
                nc.vector.tensor_mul(out=q2, in0=x2, in1=x2)

                g = gpool.tile([K, nf], f32)
                ss = spool.tile([K, nf], f32)
                for s_ in range(nsub):
                    sl = slice(s_ * 512, (s_ + 1) * 512)
                    nc.tensor.matmul(
                        out=g[:, sl],
                        lhsT=w_s[:, 0, :],
                        rhs=x1[:, sl],
                        start=True,
                        stop=False,
                    )
                    nc.tensor.matmul(
                        out=g[:, sl],
                        lhsT=w_s[:, 1, :],
                        rhs=x2[:, sl],
                        start=False,
                        stop=True,
                    )
                for s_ in range(nsub):
                    sl = slice(s_ * 512, (s_ + 1) * 512)
                    nc.tensor.matmul(
                        out=ss[:, sl],
                        lhsT=ones_s,
                        rhs=q1[:, sl],
                        start=True,
                        stop=False,
                    )
                    nc.tensor.matmul(
                        out=ss[:, sl],
                        lhsT=ones_s,
                        rhs=q2[:, sl],
                        start=False,
                        stop=True,
                    )

                r = rpool.tile([K, nf], f32, tag="r")
                nc.scalar.activation(
                    out=r, in_=ss, func=mybir.ActivationFunctionType.Sqrt
                )
                ri = rpool.tile([K, nf], f32, tag="ri")
                nc.vector.reciprocal(out=ri, in_=r)

                t = opool.tile([K, nf], f32, tag="t")
                nc.vector.tensor_mul(out=t, in0=g, in1=ri)
                u = opool.tile([K, nf], f32, tag="u")
                nc.scalar.activation(
                    out=u,
                    in_=t,
                    func=mybir.ActivationFunctionType.Sqrt,
                    bias=bv_s,
                    scale=sv_s,
                )
                o = opool.tile([K, nf], f32, tag="o")
                nc.gpsimd.tensor_scalar_mul(out=o, in0=u, scalar1=nv_s)
                nc.gpsimd.dma_start(out=out[b, :, h0 : h0 + nf], in_=o)

    return _split_excess_waits(nc) if split_waits else nc


def host_inputs(features, prototypes, distance_scale, bpc=BPC, hw=HW):
    """Build per-core input maps (host-side prep)."""
    pn = prototypes / np.maximum(
        np.sqrt(np.sum(prototypes * prototypes, axis=-1, keepdims=True)), EPS
    )
    s = abs(float(distance_scale.reshape(-1)[0]))
    # wneg2[c, j, k] = -2 * pn[k, j*128 + c]
    wneg2 = np.ascontiguousarray(
        (-2.0 * pn).T.reshape(2, 128, K).transpose(1, 0, 2)
    ).astype(np.float32)
    svec = np.full((K, 1), s * s, np.float32)
    bvec = np.full((K, 1), 2.0 * s * s, np.float32)
    negv = np.full((K, 1), -1.0, np.float32)
    import ml_dtypes
    wneg2_bf = wneg2.astype(ml_dtypes.bfloat16)

    ncores = features.shape[0] // bpc
    fr = features.reshape(ncores, bpc, C, hw)
    in_maps = []
    for i in range(ncores):
        in_maps.append(
            {
                "features": np.ascontiguousarray(fr[i]),
                "wneg2": wneg2_bf,
                "svec": svec,
                "bvec": bvec,
                "negv": negv,
            }
        )
    return in_maps


_CACHE = {}


def kernel(features, prototypes, distance_scale):
    from concourse.bass_utils import run_bass_kernel_spmd

    if "nc" not in _CACHE:
        _CACHE["nc"] = build_program()
    nc = _CACHE["nc"]
    in_maps = host_inputs(features, prototypes, distance_scale)
    res = run_bass_kernel_spmd(nc, in_maps, core_ids=list(range(NCORES)))
    outs = [res.results[i]["out"].reshape(BPC, K, H, W) for i in range(NCORES)]
    return np.concatenate(outs, axis=0).astype(np.float32)



# revision 25
# speedup vs baseline: 3.5264x; 3.5264x over previous
"""IsoMaxPlus first-part kernel for Trainium2 (8 NeuronCores, SPMD).

Math (per point n, prototype k):
    c[n,k] = (x_n . p_hat_k) / ||x_n||          (cosine sim)
    out[n,k] = -|s| * sqrt(2 - 2 c[n,k])

Device dataflow per core (2 of 16 batches, channels on partitions),
per macro-tile of NF=1024 points:
    DMA  xt [128, 2, NF] f32                     (C=256 split in 2 chunks)
    DVE  q1 = xt0^2            ACT  q2 = Square(xt1)
    PE   G[0:19]  = W.T @ x    (fp32r, W = p_hat)
         G[19:20] = ones.T @ q1 + ones.T @ q2    (ss = ||x||^2)
    DMA  compact ss [1,NF] -> [128, NF/128]      (cheap per-lane ops)
    DVE  iv = 1/ss             ACT  ri = sqrt(iv)   (= 1/||x||, compact)
    DMA  ri -> [1, NF]
    DVE  t = G[0:19] * bcast(ri)                 (= c, stride-0 partition bcast)
    ACT  u = Sqrt(-2s^2 * t + 2s^2)              (= |s| sqrt(d2))
    ACT  o = Copy(-u); DMA out

All [19,NF] / [128,NF] elementwise ops cost ~free-size cycles per lane;
the compact reshape makes the reciprocal ~free. No gpsimd compute, no
bf16 casts (fp32r matmul streams at full rate for free >= 256).
"""

import numpy as np

B, C, H, W = 16, 256, 128, 256
K = 19
NCORES = 8
BPC = B // NCORES          # batches per core
HW = H * W                 # 32768 points per batch
NF = 1024                  # points per macro-tile
EPS = 1e-12


def _split_excess_waits(nc):
    """Walrus limits the sync-wait slots per ISA instruction (TensorTensor
    takes only 1, DMAs 2, ...). Hoist excess waits onto same-engine NoOps
    inserted right before the instruction — engines execute in order, so
    all waits still complete before the instruction runs."""
    import bass_rust
    import concourse.mybir as mybir

    limits = {}
    default_limit = 1
    skip = {"InstEventSemaphore", "InstNoOp", "InstCall",
            "InstUnconditionalBranch", "InstISA", "InstRegisterMove"}
    nseq = 0
    for fn in nc.m.functions:
        for blk in fn.blocks:
            new = []
            for I in blk.instructions:
                tn = type(I).__name__
                si = I.sync_info
                waits = list(si.on_wait) if si else []
                lim = limits.get(tn, default_limit)
                if tn in skip or len(waits) <= lim:
                    new.append(I)
                    continue
                keep = waits[-lim:]
                excess = waits[:-lim]
                for w in excess:
                    nop = mybir.InstNoOp(name=f"{I.name}-w{nseq}", ins=[], outs=[])
                    nseq += 1
                    nop.engine = I.engine
                    nop.sync_info = bass_rust.SyncInfo(on_wait=[w], on_update=[])
                    new.append(nop)
                I.sync_info = bass_rust.SyncInfo(
                    on_wait=keep, on_update=list(si.on_update) if si else []
                )
                new.append(I)
            blk.instructions = new
    return nc


def build_program(bpc=BPC, hw=HW, nf=NF, split_waits=True):
    import os
    from contextlib import ExitStack

    import concourse.bass as bass
    import concourse.mybir as mybir
    import concourse.tile as tile

    variant = os.environ.get("BISECT_VARIANT", "v0")
    no_f32r = variant == "v1"      # replace fp32r matmuls with bf16 stand-ins
    no_bcast = variant == "v2"     # drop [1,K]-lhsT broadcast/accum matmuls
    own_group = variant == "v3"    # r-acc into its own PSUM tile (no mixed group)
    no_cmp = variant == "v6"       # drop compact chain + its reshape DMAs

    f32 = mybir.dt.float32
    f32r = mybir.dt.float32r
    AF = mybir.ActivationFunctionType
    nsub = nf // 512
    nmacro = hw // nf
    ncmp = nf // 128           # compact free size

    bf16 = mybir.dt.bfloat16
    nc = bass.Bass()
    # features/wproto declared float32r (same f32 bits) so the fp32r
    # matmuls see fp32r-dtype producers without any conversion step
    feat = nc.declare_dram_parameter("features", [bpc, C, hw], f32r, isOutput=False)
    wp = nc.declare_dram_parameter("wproto", [128, 2, K], f32r, isOutput=False)
    sv = nc.declare_dram_parameter("svec", [K, 1], f32, isOutput=False)
    out = nc.declare_dram_parameter("out", [bpc, K, hw], f32, isOutput=True)

    with ExitStack() as ctx:
        tc = ctx.enter_context(tile.TileContext(nc))
        singles = ctx.enter_context(tc.tile_pool(name="singles", bufs=1))
        xpool = ctx.enter_context(tc.tile_pool(name="x", bufs=4))
        qpool = ctx.enter_context(tc.tile_pool(name="q", bufs=4))
        gpool = ctx.enter_context(tc.tile_pool(name="g", bufs=2, space="PSUM"))
        spool = ctx.enter_context(tc.tile_pool(name="s", bufs=1, space="PSUM"))
        wpsum = ctx.enter_context(tc.tile_pool(name="wb", bufs=1, space="PSUM"))
        cpool = ctx.enter_context(tc.tile_pool(name="c", bufs=8))
        r1pool = ctx.enter_context(tc.tile_pool(name="r1", bufs=3))
        ypool = ctx.enter_context(tc.tile_pool(name="y", bufs=3))
        opool = ctx.enter_context(tc.tile_pool(name="o", bufs=3))

        w_s = singles.tile([128, 2, K], f32r)
        nc.sync.dma_start(out=w_s, in_=wp[:, :, :])
        wb16_s = singles.tile([128, K], bf16)
        nc.vector.memset(wb16_s, 1.0)
        ones_s = singles.tile([128, 2, 1], bf16)
        nc.vector.memset(ones_s, 1.0)
        mone_s = singles.tile([1, K], bf16)
        nc.vector.memset(mone_s, -1.0)
        sv_s = singles.tile([K, 1], f32)
        nc.sync.dma_start(out=sv_s, in_=sv[:, :])

        def late(st):
            """Deferred tail of a tile: runs one iteration later so the
            compact-chain latency (ACT copy -> DMA -> DVE -> ACT -> DMA)
            never head-of-line-blocks the PE stream."""
            G, r1, w1, b, h0 = st
            if no_bcast:
                y = ypool.tile([K, nf], f32, tag="y")
                nc.scalar.activation(out=y, in_=G, func=AF.Sqrt, scale=sv_s)
                nc.sync.dma_start(out=out[b, :, h0 : h0 + nf], in_=y)
                return
            # G := g - r  (accumulate -bcast(r) onto the dot products)
            acc_dst = G
            if own_group:
                acc_dst = wpsum.tile([K, nf], f32)
            for s_ in range(nsub):
                sl = slice(s_ * 512, (s_ + 1) * 512)
                nc.tensor.matmul(
                    out=acc_dst[:, sl],
                    lhsT=mone_s,
                    rhs=r1[:, sl],
                    start=own_group,
                    stop=True,
                    skip_group_check=True,
                )
            # Wb := bcast(-1/sqrt(r))
            Wb = wpsum.tile([K, nf], f32)
            for s_ in range(nsub):
                sl = slice(s_ * 512, (s_ + 1) * 512)
                nc.tensor.matmul(
                    out=Wb[:, sl],
                    lhsT=mone_s,
                    rhs=w1[:, sl],
                    start=True,
                    stop=True,
                    skip_group_check=True,
                )
            # y = sqrt(-2 s^2 (g - r)) = |s| sqrt(2) sqrt(r - g)
            y = ypool.tile([K, nf], f32, tag="y")
            nc.scalar.activation(out=y, in_=G, func=AF.Sqrt, scale=sv_s)
            # o = y * (-1/sqrt(r)) = -|s| sqrt(2 - 2 g/r)
            o = opool.tile([K, nf], f32, tag="o")
            nc.vector.tensor_mul(out=o, in0=y, in1=Wb)
            nc.sync.dma_start(out=out[b, :, h0 : h0 + nf], in_=o)

        pending = None
        for b in range(bpc):
            for m in range(nmacro):
                h0 = m * nf
                xt = xpool.tile([128, 2, nf], f32r, tag="xt")
                nc.sync.dma_start(
                    out=xt,
                    in_=feat[b, :, h0 : h0 + nf].rearrange(
                        "(j c) n -> c j n", c=128
                    ),
                )

                q1 = qpool.tile([128, nf], bf16, tag="q1")
                nc.vector.tensor_mul(
                    out=q1,
                    in0=xt[:, 0, :].bitcast(f32),
                    in1=xt[:, 0, :].bitcast(f32),
                )
                q2 = qpool.tile([128, nf], bf16, tag="q2")
                nc.scalar.activation(
                    out=q2, in_=xt[:, 1, :].bitcast(f32), func=AF.Square
                )

                G = gpool.tile([K, nf], f32)
                for s_ in range(nsub):
                    sl = slice(s_ * 512, (s_ + 1) * 512)
                    for j in range(2):
                        if no_f32r:
                            lhsT, rhs = wb16_s, q1[:, sl]
                        else:
                            lhsT, rhs = w_s[:, j, :], xt[:, j, sl]
                        nc.tensor.matmul(
                            out=G[:, sl],
                            lhsT=lhsT,
                            rhs=rhs,
                            start=(j == 0),
                            stop=False,
                            skip_group_check=True,
                        )
                # ss accumulates into a single PSUM row [1, nf]
                S = spool.tile([1, nf], f32)
                for s_ in range(nsub):
                    sl = slice(s_ * 512, (s_ + 1) * 512)
                    nc.tensor.matmul(
                        out=S[:, sl],
                        lhsT=ones_s[:, 0, :],
                        rhs=q1[:, sl],
                        start=True,
                        stop=False,
                        skip_group_check=True,
                    )
                    nc.tensor.matmul(
                        out=S[:, sl],
                        lhsT=ones_s[:, 1, :],
                        rhs=q2[:, sl],
                        start=False,
                        stop=True,
                        skip_group_check=True,
                    )
                # r = sqrt(ss) straight out of PSUM; this row doubles as
                # the broadcast source for the r-accumulate matmul
                r1 = r1pool.tile([1, nf], bf16, tag="r1")
                nc.scalar.activation(out=r1, in_=S, func=AF.Sqrt)
                # compact chain for w = 1/sqrt(r): [1,nf] -> [128, nf/128]
                # so recip/sqrt cost nf/128 per lane instead of nf
                w1 = r1pool.tile([1, nf], bf16, tag="w1")
                if no_cmp:
                    nc.vector.memset(w1, 1.0)
                else:
                    cmw = cpool.tile([128, ncmp], bf16, tag="cmw")
                    nc.gpsimd.dma_start(out=cmw, in_=r1)
                    rq = cpool.tile([128, ncmp], f32, tag="rq")
                    nc.scalar.activation(out=rq, in_=cmw, func=AF.Sqrt)
                    wc = cpool.tile([128, ncmp], bf16, tag="wc")
                    with nc.allow_low_precision(reason="bf16 ok: 2e-2 tol"):
                        nc.vector.reciprocal(out=wc, in_=rq)         # r^-1/2
                    nc.gpsimd.dma_start(out=w1, in_=wc)

                if pending is not None:
                    late(pending)
                pending = (G, r1, w1, b, h0)
        late(pending)

    return _split_excess_waits(nc) if split_waits else nc


def host_inputs(features, prototypes, distance_scale, bpc=BPC, hw=HW):
    """Build per-core input maps (host-side prep)."""
    pn = prototypes / np.maximum(
        np.sqrt(np.sum(prototypes * prototypes, axis=-1, keepdims=True)), EPS
    )
    s = abs(float(np.asarray(distance_scale).reshape(-1)[0]))
    # wproto[c, j, k] = pn[k, j*128 + c]
    wproto = np.ascontiguousarray(
        pn.T.reshape(2, 128, K).transpose(1, 0, 2)
    ).astype(np.float32)
    svec = np.full((K, 1), -2.0 * s * s, np.float32)

    ncores = features.shape[0] // bpc
    fr = features.reshape(ncores, bpc, C, hw)
    in_maps = []
    for i in range(ncores):
        in_maps.append(
            {
                "features": np.ascontiguousarray(fr[i]),
                "wproto": wproto,
                "svec": svec,
            }
        )
    return in_maps


_CACHE = {}


def kernel(features, prototypes, distance_scale):
    from concourse.bass_utils import run_bass_kernel_spmd

    if "nc" not in _CACHE:
        _CACHE["nc"] = build_program()
    nc = _CACHE["nc"]
    in_maps = host_inputs(features, prototypes, distance_scale)
    res = run_bass_kernel_spmd(nc, in_maps, core_ids=list(range(NCORES)))
    outs = [res.results[i]["out"].reshape(BPC, K, H, W) for i in range(NCORES)]
    return np.concatenate(outs, axis=0).astype(np.float32)


# revision 26
# speedup vs baseline: 3.8071x; 1.0796x over previous
"""IsoMaxPlus first-part kernel for Trainium2 (8 NeuronCores, SPMD).

Math (per point n, prototype k):
    c[n,k] = (x_n . p_hat_k) / ||x_n||          (cosine sim)
    out[n,k] = -|s| * sqrt(2 - 2 c[n,k])

The device computes u = |s|*sqrt(2-2c) in bf16; the host negates during
the bf16 -> f32 upcast of the gather (a sqrt's sign cannot be flipped
on-device without a whole extra engine pass).

Per macro-tile of NF=1024 points (per core: 2 of 16 batches):
    DMA  xt [128, 2, NF] f32r                   (C=256 split in 2 chunks)
    DVE  q1 = xt0^2 (bf16)     ACT  q2 = Square(xt1) (bf16)
    DVE  qs = q1 + q2 (bf16)                    (fold 256-chan ssq to 128)
    PE   G[19,NF]  = W.T @ x   (fp32r, 2 passes; W = p_hat)
         S[1,NF]   = ones.T @ qs (bf16, 1 pass; = ||x||^2)
    ACT  r1[1,NF]  = Sqrt(S)                    (PSUM extract, = ||x||)
    DMA  cm[128,8] <- r1                        (compact: 8/lane not 1024)
    DVE  ic = 1/cm                              (= 1/||x||, compact)
    DMA  rid[tile] <- ic                        (DRAM round-trip ...)
    DMA  ribc[19,NF] <- rid (stride-0 x19)      (... broadcast to 19 rows)
    DVE  t = G * ribc                           (= c)
    ACT  u = Sqrt(-2s^2 t + 2s^2) (bf16)        (= |s| sqrt(d2))
    DMA  out <- u

The tail (ribc/t/u/out) of tile m is emitted during tile m+1 so the
compact-chain latency never head-of-line-blocks any engine. PE work is
3 streaming passes/tile -- the engine floor under the ~50% power
throttle observed on these cores (PE never ramps past ~1.2 GHz).
"""

import numpy as np

B, C, H, W = 16, 256, 128, 256
K = 19
NCORES = 8
BPC = B // NCORES          # batches per core
HW = H * W                 # 32768 points per batch
NF = 1024                  # points per macro-tile
EPS = 1e-12


def _split_excess_waits(nc):
    """Walrus limits the sync-wait slots per ISA instruction (TensorTensor
    takes only 1, DMAs 2, ...). Hoist excess waits onto same-engine NoOps
    inserted right before the instruction — engines execute in order, so
    all waits still complete before the instruction runs."""
    import bass_rust
    import concourse.mybir as mybir

    limits = {}
    default_limit = 1
    skip = {"InstEventSemaphore", "InstNoOp", "InstCall",
            "InstUnconditionalBranch", "InstISA", "InstRegisterMove"}
    nseq = 0
    for fn in nc.m.functions:
        for blk in fn.blocks:
            new = []
            for I in blk.instructions:
                tn = type(I).__name__
                si = I.sync_info
                waits = list(si.on_wait) if si else []
                lim = limits.get(tn, default_limit)
                if tn in skip or len(waits) <= lim:
                    new.append(I)
                    continue
                keep = waits[-lim:]
                excess = waits[:-lim]
                for w in excess:
                    nop = mybir.InstNoOp(name=f"{I.name}-w{nseq}", ins=[], outs=[])
                    nseq += 1
                    nop.engine = I.engine
                    nop.sync_info = bass_rust.SyncInfo(on_wait=[w], on_update=[])
                    new.append(nop)
                I.sync_info = bass_rust.SyncInfo(
                    on_wait=keep, on_update=list(si.on_update) if si else []
                )
                new.append(I)
            blk.instructions = new
    return nc


def build_program(bpc=BPC, hw=HW, nf=NF, split_waits=True):
    from contextlib import ExitStack

    import concourse.bass as bass
    import concourse.mybir as mybir
    import concourse.tile as tile

    f32 = mybir.dt.float32
    f32r = mybir.dt.float32r
    bf16 = mybir.dt.bfloat16
    AF = mybir.ActivationFunctionType
    nsub = nf // 512
    nmacro = hw // nf
    ncmp = nf // 128           # compact free size
    ntiles = bpc * nmacro

    nc = bass.Bass()
    # features/wproto declared float32r (same f32 bits) so the fp32r
    # matmuls see fp32r-dtype producers without any conversion step
    feat = nc.declare_dram_parameter("features", [bpc, C, hw], f32r, isOutput=False)
    wp = nc.declare_dram_parameter("wproto", [128, 2, K], f32r, isOutput=False)
    sv = nc.declare_dram_parameter("svec", [K, 1], f32, isOutput=False)
    bv = nc.declare_dram_parameter("bvec", [K, 1], f32, isOutput=False)
    out = nc.declare_dram_parameter("out", [bpc, K, hw], bf16, isOutput=True)
    rid = nc.dram_tensor("ridscratch", (ntiles, nf), bf16, kind="Internal")

    with ExitStack() as ctx:
        tc = ctx.enter_context(tile.TileContext(nc))
        singles = ctx.enter_context(tc.tile_pool(name="singles", bufs=1))
        xpool = ctx.enter_context(tc.tile_pool(name="x", bufs=4))
        qpool = ctx.enter_context(tc.tile_pool(name="q", bufs=4))
        gpool = ctx.enter_context(tc.tile_pool(name="g", bufs=2, space="PSUM"))
        spool = ctx.enter_context(tc.tile_pool(name="s", bufs=2, space="PSUM"))
        cpool = ctx.enter_context(tc.tile_pool(name="c", bufs=6))
        r1pool = ctx.enter_context(tc.tile_pool(name="r1", bufs=3))
        bpool = ctx.enter_context(tc.tile_pool(name="bc", bufs=3))
        tpool = ctx.enter_context(tc.tile_pool(name="t", bufs=3))
        opool = ctx.enter_context(tc.tile_pool(name="o", bufs=3))

        w_s = singles.tile([128, 2, K], f32r)
        nc.sync.dma_start(out=w_s, in_=wp[:, :, :])
        ones_s = singles.tile([128, 1], bf16)
        nc.vector.memset(ones_s, 1.0)
        sv_s = singles.tile([K, 1], f32)
        nc.sync.dma_start(out=sv_s, in_=sv[:, :])
        bv_s = singles.tile([K, 1], f32)
        nc.sync.dma_start(out=bv_s, in_=bv[:, :])

        def late(st):
            """Deferred tail of a tile: emitted one iteration later so the
            compact-chain latency never head-of-line-blocks an engine."""
            G, idx, b, h0 = st
            ribc = bpool.tile([K, nf], bf16, tag="ribc")
            nc.gpsimd.dma_start(
                out=ribc,
                in_=rid.ap()[idx : idx + 1, :].partition_broadcast(K).squeeze(1),
            )
            t = tpool.tile([K, nf], f32, tag="t")
            nc.vector.tensor_mul(out=t, in0=G, in1=ribc)
            u = opool.tile([K, nf], bf16, tag="u")
            nc.scalar.activation(
                out=u, in_=t, func=AF.Sqrt, bias=bv_s, scale=sv_s
            )
            nc.sync.dma_start(out=out[b, :, h0 : h0 + nf], in_=u)

        pending = None
        for b in range(bpc):
            for m in range(nmacro):
                h0 = m * nf
                idx = b * nmacro + m
                xt = xpool.tile([128, 2, nf], f32r, tag="xt")
                nc.sync.dma_start(
                    out=xt,
                    in_=feat[b, :, h0 : h0 + nf].rearrange(
                        "(j c) n -> c j n", c=128
                    ),
                )

                q1 = qpool.tile([128, nf], bf16, tag="q1")
                nc.vector.tensor_mul(
                    out=q1,
                    in0=xt[:, 0, :].bitcast(f32),
                    in1=xt[:, 0, :].bitcast(f32),
                )
                q2 = qpool.tile([128, nf], bf16, tag="q2")
                nc.scalar.activation(
                    out=q2, in_=xt[:, 1, :].bitcast(f32), func=AF.Square
                )
                qs = qpool.tile([128, nf], bf16, tag="qs")
                nc.vector.tensor_add(out=qs, in0=q1, in1=q2)

                G = gpool.tile([K, nf], f32)
                for s_ in range(nsub):
                    sl = slice(s_ * 512, (s_ + 1) * 512)
                    nc.tensor.matmul(
                        out=G[:, sl],
                        lhsT=w_s[:, 0, :],
                        rhs=xt[:, 0, sl],
                        start=True,
                        stop=False,
                    )
                    nc.tensor.matmul(
                        out=G[:, sl],
                        lhsT=w_s[:, 1, :],
                        rhs=xt[:, 1, sl],
                        start=False,
                        stop=True,
                    )
                S = spool.tile([1, nf], f32)
                for s_ in range(nsub):
                    sl = slice(s_ * 512, (s_ + 1) * 512)
                    nc.tensor.matmul(
                        out=S[:, sl],
                        lhsT=ones_s,
                        rhs=qs[:, sl],
                        start=True,
                        stop=True,
                    )
                # r = sqrt(ss) straight out of PSUM ([1,nf]: 1 lane)
                r1 = r1pool.tile([1, nf], bf16, tag="r1")
                nc.scalar.activation(out=r1, in_=S, func=AF.Sqrt)
                # compact so the reciprocal costs nf/128 per lane, then
                # DRAM round-trip to broadcast 1/r across the 19 rows
                cm = cpool.tile([128, ncmp], bf16, tag="cm")
                nc.gpsimd.dma_start(out=cm, in_=r1)
                ic = cpool.tile([128, ncmp], bf16, tag="ic")
                with nc.allow_low_precision(reason="bf16 ok: 2e-2 rel tol"):
                    nc.vector.reciprocal(out=ic, in_=cm)
                nc.gpsimd.dma_start(out=rid.ap()[idx : idx + 1, :], in_=ic)

                if pending is not None:
                    late(pending)
                pending = (G, idx, b, h0)
        late(pending)

    return _split_excess_waits(nc) if split_waits else nc


def host_inputs(features, prototypes, distance_scale, bpc=BPC, hw=HW):
    """Build per-core input maps (host-side prep)."""
    pn = prototypes / np.maximum(
        np.sqrt(np.sum(prototypes * prototypes, axis=-1, keepdims=True)), EPS
    )
    s = abs(float(np.asarray(distance_scale).reshape(-1)[0]))
    # wproto[c, j, k] = pn[k, j*128 + c]
    wproto = np.ascontiguousarray(
        pn.T.reshape(2, 128, K).transpose(1, 0, 2)
    ).astype(np.float32)
    svec = np.full((K, 1), -2.0 * s * s, np.float32)
    bvec = np.full((K, 1), 2.0 * s * s, np.float32)

    ncores = features.shape[0] // bpc
    fr = features.reshape(ncores, bpc, C, hw)
    in_maps = []
    for i in range(ncores):
        in_maps.append(
            {
                "features": np.ascontiguousarray(fr[i]),
                "wproto": wproto,
                "svec": svec,
                "bvec": bvec,
            }
        )
    return in_maps


_CACHE = {}


def kernel(features, prototypes, distance_scale):
    from concourse.bass_utils import run_bass_kernel_spmd

    if "nc" not in _CACHE:
        _CACHE["nc"] = build_program()
    nc = _CACHE["nc"]
    in_maps = host_inputs(features, prototypes, distance_scale)
    res = run_bass_kernel_spmd(nc, in_maps, core_ids=list(range(NCORES)))
    out = np.empty((NCORES, BPC, K, H, W), np.float32)
    for i in range(NCORES):
        # device returns u = |s| sqrt(d2) in bf16; negate during upcast
        np.multiply(
            res.results[i]["out"].reshape(BPC, K, H, W).astype(np.float32),
            -1.0,
            out=out[i],
        )
    return out.reshape(B, K, H, W)


# revision 27
# speedup vs baseline: 4.0466x; 1.0629x over previous
"""IsoMaxPlus first-part kernel for Trainium2 (8 NeuronCores, SPMD).

Math (per point n, prototype k):
    c[n,k] = (x_n . p_hat_k) / ||x_n||          (cosine sim)
    out[n,k] = -|s| * sqrt(2 - 2 c[n,k])

The device computes u = |s|*sqrt(2-2c) in bf16; the host negates during
the bf16 -> f32 upcast of the gather (a sqrt's sign cannot be flipped
on-device without a whole extra engine pass).

Per macro-tile of NF=1024 points (per core: 2 of 16 batches):
    DMA  xt [128, 2, NF] f32r                   (C=256 split in 2 chunks)
    DVE  q1 = xt0^2 (bf16)     ACT  q2 = Square(xt1) (bf16)
    DVE  qs = q1 + q2 (bf16)                    (fold 256-chan ssq to 128)
    PE   G[19,NF]  = W.T @ x   (fp32r, 2 passes; W = p_hat)
         S[1,NF]   = ones.T @ qs (bf16, 1 pass; = ||x||^2)
    ACT  r1[1,NF]  = Sqrt(S)                    (PSUM extract, = ||x||)
    DMA  cm[128,8] <- r1                        (compact: 8/lane not 1024)
    DVE  ic = 1/cm                              (= 1/||x||, compact)
    DMA  rid[tile] <- ic                        (DRAM round-trip ...)
    DMA  ribc[19,NF] <- rid (stride-0 x19)      (... broadcast to 19 rows)
    DVE  t = G * ribc                           (= c)
    ACT  u = Sqrt(-2s^2 t + 2s^2) (bf16)        (= |s| sqrt(d2))
    DMA  out <- u

The tail (ribc/t/u/out) of tile m is emitted during tile m+1 so the
compact-chain latency never head-of-line-blocks any engine. PE work is
3 streaming passes/tile -- the engine floor under the ~50% power
throttle observed on these cores (PE never ramps past ~1.2 GHz).
"""

import numpy as np

B, C, H, W = 16, 256, 128, 256
K = 19
NCORES = 8
BPC = B // NCORES          # batches per core
HW = H * W                 # 32768 points per batch
NF = 1024                  # points per macro-tile
EPS = 1e-12


def _split_excess_waits(nc):
    """Walrus limits the sync-wait slots per ISA instruction (TensorTensor
    takes only 1, DMAs 2, ...). Hoist excess waits onto same-engine NoOps
    inserted right before the instruction — engines execute in order, so
    all waits still complete before the instruction runs."""
    import bass_rust
    import concourse.mybir as mybir

    limits = {}
    default_limit = 1
    skip = {"InstEventSemaphore", "InstNoOp", "InstCall",
            "InstUnconditionalBranch", "InstISA", "InstRegisterMove"}
    nseq = 0
    for fn in nc.m.functions:
        for blk in fn.blocks:
            new = []
            for I in blk.instructions:
                tn = type(I).__name__
                si = I.sync_info
                waits = list(si.on_wait) if si else []
                lim = limits.get(tn, default_limit)
                if tn in skip or len(waits) <= lim:
                    new.append(I)
                    continue
                keep = waits[-lim:]
                excess = waits[:-lim]
                for w in excess:
                    nop = mybir.InstNoOp(name=f"{I.name}-w{nseq}", ins=[], outs=[])
                    nseq += 1
                    nop.engine = I.engine
                    nop.sync_info = bass_rust.SyncInfo(on_wait=[w], on_update=[])
                    new.append(nop)
                I.sync_info = bass_rust.SyncInfo(
                    on_wait=keep, on_update=list(si.on_update) if si else []
                )
                new.append(I)
            blk.instructions = new
    return nc


def build_program(bpc=BPC, hw=HW, nf=NF, split_waits=True):
    from contextlib import ExitStack

    import concourse.bass as bass
    import concourse.mybir as mybir
    import concourse.tile as tile

    f32 = mybir.dt.float32
    f32r = mybir.dt.float32r
    bf16 = mybir.dt.bfloat16
    AF = mybir.ActivationFunctionType
    nsub = nf // 512
    nmacro = hw // nf
    ncmp = nf // 128           # compact free size
    ntiles = bpc * nmacro

    nc = bass.Bass()
    # features/wproto declared float32r (same f32 bits) so the fp32r
    # matmuls see fp32r-dtype producers without any conversion step
    feat = nc.declare_dram_parameter("features", [bpc, C, hw], f32r, isOutput=False)
    wp = nc.declare_dram_parameter("wproto", [128, 2, K], f32r, isOutput=False)
    sv = nc.declare_dram_parameter("svec", [K, 1], f32, isOutput=False)
    bv = nc.declare_dram_parameter("bvec", [K, 1], f32, isOutput=False)
    out = nc.declare_dram_parameter("out", [bpc, K, hw], bf16, isOutput=True)
    rid = nc.dram_tensor("ridscratch", (ntiles, nf), bf16, kind="Internal")

    with ExitStack() as ctx:
        tc = ctx.enter_context(tile.TileContext(nc))
        singles = ctx.enter_context(tc.tile_pool(name="singles", bufs=1))
        xpool = ctx.enter_context(tc.tile_pool(name="x", bufs=4))
        qpool = ctx.enter_context(tc.tile_pool(name="q", bufs=4))
        gpool = ctx.enter_context(tc.tile_pool(name="g", bufs=3, space="PSUM"))
        spool = ctx.enter_context(tc.tile_pool(name="s", bufs=1, space="PSUM"))
        cpool = ctx.enter_context(tc.tile_pool(name="c", bufs=8))
        r1pool = ctx.enter_context(tc.tile_pool(name="r1", bufs=4))
        bpool = ctx.enter_context(tc.tile_pool(name="bc", bufs=3))
        tpool = ctx.enter_context(tc.tile_pool(name="t", bufs=3))
        opool = ctx.enter_context(tc.tile_pool(name="o", bufs=3))

        w_s = singles.tile([128, 2, K], f32r)
        nc.sync.dma_start(out=w_s, in_=wp[:, :, :])
        ones_s = singles.tile([128, 1], bf16)
        nc.vector.memset(ones_s, 1.0)
        sv_s = singles.tile([K, 1], f32)
        nc.sync.dma_start(out=sv_s, in_=sv[:, :])
        bv_s = singles.tile([K, 1], f32)
        nc.sync.dma_start(out=bv_s, in_=bv[:, :])

        def late(st):
            """Deferred tail of a tile: emitted one iteration later so the
            compact-chain latency never head-of-line-blocks an engine."""
            G, idx, b, h0 = st
            ribc = bpool.tile([K, nf], bf16, tag="ribc")
            nc.gpsimd.dma_start(
                out=ribc,
                in_=rid.ap()[idx : idx + 1, :].partition_broadcast(K).squeeze(1),
            )
            t = tpool.tile([K, nf], f32, tag="t")
            nc.vector.tensor_mul(out=t, in0=G, in1=ribc)
            u = opool.tile([K, nf], bf16, tag="u")
            nc.scalar.activation(
                out=u, in_=t, func=AF.Sqrt, bias=bv_s, scale=sv_s
            )
            nc.sync.dma_start(out=out[b, :, h0 : h0 + nf], in_=u)

        pending = []
        for b in range(bpc):
            for m in range(nmacro):
                h0 = m * nf
                idx = b * nmacro + m
                xt = xpool.tile([128, 2, nf], f32r, tag="xt")
                nc.sync.dma_start(
                    out=xt,
                    in_=feat[b, :, h0 : h0 + nf].rearrange(
                        "(j c) n -> c j n", c=128
                    ),
                )

                q1 = qpool.tile([128, nf], bf16, tag="q1")
                nc.vector.tensor_mul(
                    out=q1,
                    in0=xt[:, 0, :].bitcast(f32),
                    in1=xt[:, 0, :].bitcast(f32),
                )
                q2 = qpool.tile([128, nf], bf16, tag="q2")
                nc.scalar.activation(
                    out=q2, in_=xt[:, 1, :].bitcast(f32), func=AF.Square
                )
                qs = qpool.tile([128, nf], bf16, tag="qs")
                nc.vector.tensor_add(out=qs, in0=q1, in1=q2)

                G = gpool.tile([K, nf], f32)
                for s_ in range(nsub):
                    sl = slice(s_ * 512, (s_ + 1) * 512)
                    nc.tensor.matmul(
                        out=G[:, sl],
                        lhsT=w_s[:, 0, :],
                        rhs=xt[:, 0, sl],
                        start=True,
                        stop=False,
                    )
                    nc.tensor.matmul(
                        out=G[:, sl],
                        lhsT=w_s[:, 1, :],
                        rhs=xt[:, 1, sl],
                        start=False,
                        stop=True,
                    )
                S = spool.tile([1, nf], f32)
                for s_ in range(nsub):
                    sl = slice(s_ * 512, (s_ + 1) * 512)
                    nc.tensor.matmul(
                        out=S[:, sl],
                        lhsT=ones_s,
                        rhs=qs[:, sl],
                        start=True,
                        stop=True,
                    )
                # r = sqrt(ss) straight out of PSUM ([1,nf]: 1 lane)
                r1 = r1pool.tile([1, nf], bf16, tag="r1")
                nc.scalar.activation(out=r1, in_=S, func=AF.Sqrt)
                # compact so the reciprocal costs nf/128 per lane, then
                # DRAM round-trip to broadcast 1/r across the 19 rows
                cm = cpool.tile([128, ncmp], bf16, tag="cm")
                nc.gpsimd.dma_start(out=cm, in_=r1)
                ic = cpool.tile([128, ncmp], bf16, tag="ic")
                with nc.allow_low_precision(reason="bf16 ok: 2e-2 rel tol"):
                    nc.vector.reciprocal(out=ic, in_=cm)
                nc.gpsimd.dma_start(out=rid.ap()[idx : idx + 1, :], in_=ic)

                if len(pending) == 2:
                    late(pending.pop(0))
                pending.append((G, idx, b, h0))
        for st in pending:
            late(st)

    return _split_excess_waits(nc) if split_waits else nc


def host_inputs(features, prototypes, distance_scale, bpc=BPC, hw=HW):
    """Build per-core input maps (host-side prep)."""
    pn = prototypes / np.maximum(
        np.sqrt(np.sum(prototypes * prototypes, axis=-1, keepdims=True)), EPS
    )
    s = abs(float(np.asarray(distance_scale).reshape(-1)[0]))
    # wproto[c, j, k] = pn[k, j*128 + c]
    wproto = np.ascontiguousarray(
        pn.T.reshape(2, 128, K).transpose(1, 0, 2)
    ).astype(np.float32)
    svec = np.full((K, 1), -2.0 * s * s, np.float32)
    bvec = np.full((K, 1), 2.0 * s * s, np.float32)

    ncores = features.shape[0] // bpc
    fr = features.reshape(ncores, bpc, C, hw)
    in_maps = []
    for i in range(ncores):
        in_maps.append(
            {
                "features": np.ascontiguousarray(fr[i]),
                "wproto": wproto,
                "svec": svec,
                "bvec": bvec,
            }
        )
    return in_maps


_CACHE = {}


def kernel(features, prototypes, distance_scale):
    from concourse.bass_utils import run_bass_kernel_spmd

    if "nc" not in _CACHE:
        _CACHE["nc"] = build_program()
    nc = _CACHE["nc"]
    in_maps = host_inputs(features, prototypes, distance_scale)
    res = run_bass_kernel_spmd(nc, in_maps, core_ids=list(range(NCORES)))
    out = np.empty((NCORES, BPC, K, H, W), np.float32)
    for i in range(NCORES):
        # device returns u = |s| sqrt(d2) in bf16; negate during upcast
        np.multiply(
            res.results[i]["out"].reshape(BPC, K, H, W).astype(np.float32),
            -1.0,
            out=out[i],
        )
    return out.reshape(B, K, H, W)


# revision 28
# speedup vs baseline: 4.7450x; 1.1726x over previous
"""IsoMaxPlus first-part kernel for Trainium2 (8 NeuronCores, SPMD).

Math (per point n, prototype k):
    c[n,k] = (x_n . p_hat_k) / ||x_n||          (cosine sim)
    out[n,k] = -|s| * sqrt(2 - 2 c[n,k])

The device computes u = |s|*sqrt(2-2c) in bf16; the host negates during
the bf16 -> f32 upcast of the gather (a sqrt's sign cannot be flipped
on-device without a whole extra engine pass).

Per macro-tile of NF=1024 points (per core: 2 of 16 batches):
    DMA  xt [128, 2, NF] f32r                   (C=256 split in 2 chunks)
    DVE  q1 = xt0^2 (bf16)     ACT  q2 = Square(xt1) (bf16)
    DVE  qs = q1 + q2 (bf16)                    (fold 256-chan ssq to 128)
    PE   G[19,NF]  = W.T @ x   (fp32r, 2 passes; W = p_hat)
         S[1,NF]   = ones.T @ qs (bf16, 1 pass; = ||x||^2)
    ACT  r1[1,NF]  = Sqrt(S)                    (PSUM extract, = ||x||)
    DMA  cm[128,8] <- r1                        (compact: 8/lane not 1024)
    DVE  ic = 1/cm                              (= 1/||x||, compact)
    DMA  rid[tile] <- ic                        (DRAM round-trip ...)
    DMA  ribc[19,NF] <- rid (stride-0 x19)      (... broadcast to 19 rows)
    DVE  t = G * ribc                           (= c)
    ACT  u = Sqrt(-2s^2 t + 2s^2) (bf16)        (= |s| sqrt(d2))
    DMA  out <- u

The tail (ribc/t/u/out) of tile m is emitted during tile m+1 so the
compact-chain latency never head-of-line-blocks any engine. PE work is
3 streaming passes/tile -- the engine floor under the ~50% power
throttle observed on these cores (PE never ramps past ~1.2 GHz).
"""

import numpy as np

B, C, H, W = 16, 256, 128, 256
K = 19
NCORES = 8
BPC = B // NCORES          # batches per core
HW = H * W                 # 32768 points per batch
NF = 1024                  # points per macro-tile
EPS = 1e-12


def _split_excess_waits(nc):
    """Walrus limits the sync-wait slots per ISA instruction (TensorTensor
    takes only 1, DMAs 2, ...). Hoist excess waits onto same-engine NoOps
    inserted right before the instruction — engines execute in order, so
    all waits still complete before the instruction runs."""
    import bass_rust
    import concourse.mybir as mybir

    limits = {}
    default_limit = 1
    skip = {"InstEventSemaphore", "InstNoOp", "InstCall",
            "InstUnconditionalBranch", "InstISA", "InstRegisterMove"}
    nseq = 0
    for fn in nc.m.functions:
        for blk in fn.blocks:
            new = []
            for I in blk.instructions:
                tn = type(I).__name__
                si = I.sync_info
                waits = list(si.on_wait) if si else []
                lim = limits.get(tn, default_limit)
                if tn in skip or len(waits) <= lim:
                    new.append(I)
                    continue
                keep = waits[-lim:]
                excess = waits[:-lim]
                for w in excess:
                    nop = mybir.InstNoOp(name=f"{I.name}-w{nseq}", ins=[], outs=[])
                    nseq += 1
                    nop.engine = I.engine
                    nop.sync_info = bass_rust.SyncInfo(on_wait=[w], on_update=[])
                    new.append(nop)
                I.sync_info = bass_rust.SyncInfo(
                    on_wait=keep, on_update=list(si.on_update) if si else []
                )
                new.append(I)
            blk.instructions = new
    return nc


def build_program(bpc=BPC, hw=HW, nf=NF, split_waits=True):
    from contextlib import ExitStack

    import concourse.bass as bass
    import concourse.mybir as mybir
    import concourse.tile as tile

    f32 = mybir.dt.float32
    f32r = mybir.dt.float32r
    bf16 = mybir.dt.bfloat16
    AF = mybir.ActivationFunctionType
    nsub = nf // 512
    nmacro = hw // nf
    ncmp = nf // 128           # compact free size
    ntiles = bpc * nmacro

    nc = bass.Bass()
    # features/wproto declared float32r (same f32 bits) so the fp32r
    # matmuls see fp32r-dtype producers without any conversion step
    feat = nc.declare_dram_parameter("features", [bpc, C, hw], f32r, isOutput=False)
    wp = nc.declare_dram_parameter("wproto", [128, 2, K], f32r, isOutput=False)
    sv = nc.declare_dram_parameter("svec", [K, 1], f32, isOutput=False)
    bv = nc.declare_dram_parameter("bvec", [K, 1], f32, isOutput=False)
    out = nc.declare_dram_parameter("out", [bpc, K, hw], bf16, isOutput=True)
    rid = nc.dram_tensor("ridscratch", (ntiles, nf), bf16, kind="Internal")

    with ExitStack() as ctx:
        tc = ctx.enter_context(tile.TileContext(nc))
        singles = ctx.enter_context(tc.tile_pool(name="singles", bufs=1))
        xpool = ctx.enter_context(tc.tile_pool(name="x", bufs=8))
        qpool = ctx.enter_context(tc.tile_pool(name="q", bufs=6))
        gpool = ctx.enter_context(tc.tile_pool(name="g", bufs=3, space="PSUM"))
        spool = ctx.enter_context(tc.tile_pool(name="s", bufs=1, space="PSUM"))
        cpool = ctx.enter_context(tc.tile_pool(name="c", bufs=10))
        r1pool = ctx.enter_context(tc.tile_pool(name="r1", bufs=6))
        bpool = ctx.enter_context(tc.tile_pool(name="bc", bufs=4))
        tpool = ctx.enter_context(tc.tile_pool(name="t", bufs=4))
        opool = ctx.enter_context(tc.tile_pool(name="o", bufs=4))

        w_s = singles.tile([128, 2, K], f32r)
        nc.sync.dma_start(out=w_s, in_=wp[:, :, :])
        ones_s = singles.tile([128, 1], bf16)
        nc.vector.memset(ones_s, 1.0)
        sv_s = singles.tile([K, 1], f32)
        nc.sync.dma_start(out=sv_s, in_=sv[:, :])
        bv_s = singles.tile([K, 1], f32)
        nc.sync.dma_start(out=bv_s, in_=bv[:, :])

        def late(st):
            """Deferred tail of a tile: emitted one iteration later so the
            compact-chain latency never head-of-line-blocks an engine."""
            G, idx, b, h0 = st
            ribc = bpool.tile([K, nf], bf16, tag="ribc")
            nc.gpsimd.dma_start(
                out=ribc,
                in_=rid.ap()[idx : idx + 1, :].partition_broadcast(K).squeeze(1),
            )
            t = tpool.tile([K, nf], f32, tag="t")
            nc.vector.tensor_mul(out=t, in0=G, in1=ribc)
            u = opool.tile([K, nf], bf16, tag="u")
            nc.scalar.activation(
                out=u, in_=t, func=AF.Sqrt, bias=bv_s, scale=sv_s
            )
            nc.gpsimd.dma_start(out=out[b, :, h0 : h0 + nf], in_=u)

        pending = []
        for b in range(bpc):
            for m in range(nmacro):
                h0 = m * nf
                idx = b * nmacro + m
                xt = xpool.tile([128, 2, nf], f32r, tag="xt")
                nc.sync.dma_start(
                    out=xt,
                    in_=feat[b, :, h0 : h0 + nf].rearrange(
                        "(j c) n -> c j n", c=128
                    ),
                )

                q1 = qpool.tile([128, nf], bf16, tag="q1")
                nc.vector.tensor_mul(
                    out=q1,
                    in0=xt[:, 0, :].bitcast(f32),
                    in1=xt[:, 0, :].bitcast(f32),
                )
                q2 = qpool.tile([128, nf], bf16, tag="q2")
                nc.scalar.activation(
                    out=q2, in_=xt[:, 1, :].bitcast(f32), func=AF.Square
                )
                qs = qpool.tile([128, nf], bf16, tag="qs")
                nc.vector.tensor_add(out=qs, in0=q1, in1=q2)

                G = gpool.tile([K, nf], f32)
                for s_ in range(nsub):
                    sl = slice(s_ * 512, (s_ + 1) * 512)
                    nc.tensor.matmul(
                        out=G[:, sl],
                        lhsT=w_s[:, 0, :],
                        rhs=xt[:, 0, sl],
                        start=True,
                        stop=False,
                    )
                    nc.tensor.matmul(
                        out=G[:, sl],
                        lhsT=w_s[:, 1, :],
                        rhs=xt[:, 1, sl],
                        start=False,
                        stop=True,
                    )
                S = spool.tile([1, nf], f32)
                for s_ in range(nsub):
                    sl = slice(s_ * 512, (s_ + 1) * 512)
                    nc.tensor.matmul(
                        out=S[:, sl],
                        lhsT=ones_s,
                        rhs=qs[:, sl],
                        start=True,
                        stop=True,
                    )
                # r = sqrt(ss) straight out of PSUM ([1,nf]: 1 lane)
                r1 = r1pool.tile([1, nf], bf16, tag="r1")
                nc.scalar.activation(out=r1, in_=S, func=AF.Sqrt)
                # compact so the reciprocal costs nf/128 per lane, then
                # DRAM round-trip to broadcast 1/r across the 19 rows
                cm = cpool.tile([128, ncmp], bf16, tag="cm")
                nc.gpsimd.dma_start(out=cm, in_=r1)
                ic = cpool.tile([128, ncmp], bf16, tag="ic")
                with nc.allow_low_precision(reason="bf16 ok: 2e-2 rel tol"):
                    nc.vector.reciprocal(out=ic, in_=cm)
                nc.gpsimd.dma_start(out=rid.ap()[idx : idx + 1, :], in_=ic)

                if len(pending) == 2:
                    late(pending.pop(0))
                pending.append((G, idx, b, h0))
        for st in pending:
            late(st)

    return _split_excess_waits(nc) if split_waits else nc


def host_inputs(features, prototypes, distance_scale, bpc=BPC, hw=HW):
    """Build per-core input maps (host-side prep)."""
    pn = prototypes / np.maximum(
        np.sqrt(np.sum(prototypes * prototypes, axis=-1, keepdims=True)), EPS
    )
    s = abs(float(np.asarray(distance_scale).reshape(-1)[0]))
    # wproto[c, j, k] = pn[k, j*128 + c]
    wproto = np.ascontiguousarray(
        pn.T.reshape(2, 128, K).transpose(1, 0, 2)
    ).astype(np.float32)
    svec = np.full((K, 1), -2.0 * s * s, np.float32)
    bvec = np.full((K, 1), 2.0 * s * s, np.float32)

    ncores = features.shape[0] // bpc
    fr = features.reshape(ncores, bpc, C, hw)
    in_maps = []
    for i in range(ncores):
        in_maps.append(
            {
                "features": np.ascontiguousarray(fr[i]),
                "wproto": wproto,
                "svec": svec,
                "bvec": bvec,
            }
        )
    return in_maps


_CACHE = {}


def kernel(features, prototypes, distance_scale):
    from concourse.bass_utils import run_bass_kernel_spmd

    if "nc" not in _CACHE:
        _CACHE["nc"] = build_program()
    nc = _CACHE["nc"]
    in_maps = host_inputs(features, prototypes, distance_scale)
    res = run_bass_kernel_spmd(nc, in_maps, core_ids=list(range(NCORES)))
    out = np.empty((NCORES, BPC, K, H, W), np.float32)
    for i in range(NCORES):
        # device returns u = |s| sqrt(d2) in bf16; negate during upcast
        np.multiply(
            res.results[i]["out"].reshape(BPC, K, H, W).astype(np.float32),
            -1.0,
            out=out[i],
        )
    return out.reshape(B, K, H, W)
